# revision 1
# baseline (speedup 1.0000x reference)
"""Trainium2 Bass kernel for NeuralECMModel (gnn_message_passing).

Computation (per node n):
  ent  = entity_emb @ Wp.T + bp                                  [N,50]
  node = einsum('ni,oij,nj->no', q, Wbil, ent) + bbil            [N,50]
  wtext= sum_k scores[n,k]*nbr[n,k,:] + scores[n,63]*node[n,:]   [N,50]
  agg  = wtext @ Wg.T                                            [N,50]
  feats= elu(agg + g_bias)
  out  = feats @ Wr.T + br                                       [N,1]

Sharding: pure data parallel over nodes, N=20000 -> 2500 nodes/core x 8.

Device mapping per 128-node tile:
  - PE: entity projection (host-transposed entity as stationary operand),
    bilinear stage 1 (U = qT.T @ W2 with W2[i,(o,j)]=Wbil[o,i,j]),
    wtext transpose, Wg matmul, Wr head matmul.
  - DVE: neighbor score-multiply + k-reduce, bilinear stage 2
    (U*ent j-reduce), small fixups, ELU tail assists.
  - GPSIMD: score broadcast-expansion (1-input, line rate).
  - ACT: PSUM->SBUF moves, exp/relu for ELU.
"""

import numpy as np
import ml_dtypes

import concourse.bass as bass
import concourse.bacc as bacc
import concourse.tile as tile
import concourse.mybir as mybir
from concourse.bass_utils import run_bass_kernel_spmd
from concourse.masks import make_identity

F32 = mybir.dt.float32
BF16 = mybir.dt.bfloat16
AX = mybir.AxisListType
OP = mybir.AluOpType
AF = mybir.ActivationFunctionType

N_CORES = 8
N = 20000
NLOC = N // N_CORES  # 2500
K = 63
D = 50
E = 300
OJ = D * D  # 2500
P = 128
N_TILES = (NLOC + P - 1) // P  # 20
NBR_DT = BF16  # dtype for neighbor embeddings + scores on device
PRODB_BUFS = 3
PRODN_BUFS = 3
PSU_BUFS = 2

_CACHE = {}


def _np_dt(dt):
    return ml_dtypes.bfloat16 if dt == BF16 else np.float32


def build_program(br_val: float, nbr_pl=0, bil_pl=0, repeat=1, ds=42, os_=46, smalls_pool=True):
    nc = bacc.Bacc("TRN2", debug=False, num_devices=N_CORES)
    nbr_dt = NBR_DT

    # Per-core inputs (host pre-laid-out)
    t_nbr = nc.dram_tensor("nbr", [NLOC, K * D], nbr_dt, kind="ExternalInput")
    t_sc = nc.dram_tensor("scores", [NLOC, K + 1], nbr_dt, kind="ExternalInput")
    t_sc63 = nc.dram_tensor("s63", [NLOC, 1], F32, kind="ExternalInput")
    t_qT = nc.dram_tensor("qT", [D, NLOC], BF16, kind="ExternalInput")
    t_entT0 = nc.dram_tensor("entT0", [128, NLOC], F32, kind="ExternalInput")
    t_entT1 = nc.dram_tensor("entT1", [128, NLOC], F32, kind="ExternalInput")
    t_entT2 = nc.dram_tensor("entT2", [45, NLOC], F32, kind="ExternalInput")
    # Replicated weights
    t_W2 = nc.dram_tensor("W2", [D, OJ], BF16, kind="ExternalInput")
    t_WpT0 = nc.dram_tensor("WpT0", [128, D], F32, kind="ExternalInput")
    t_WpT1 = nc.dram_tensor("WpT1", [128, D], F32, kind="ExternalInput")
    t_WpT2 = nc.dram_tensor("WpT2", [45, D], F32, kind="ExternalInput")
    t_WgT = nc.dram_tensor("WgT", [D, D], F32, kind="ExternalInput")
    t_WrT = nc.dram_tensor("WrT", [D, 1], F32, kind="ExternalInput")
    t_bbil = nc.dram_tensor("bbil_rep", [P, D], F32, kind="ExternalInput")
    t_gb = nc.dram_tensor("gbias_col", [D, 1], F32, kind="ExternalInput")
    t_out = nc.dram_tensor("out", [NLOC, 1], F32, kind="ExternalOutput")

    with tile.TileContext(nc) as tc:
        with (
            tc.tile_pool(name="res", bufs=1) as res,
            tc.tile_pool(name="nbrp", bufs=4) as nbrp,
            tc.tile_pool(name="scp", bufs=4) as scp,
            tc.tile_pool(name="prodn", bufs=PRODN_BUFS) as prodnp,
            tc.tile_pool(name="prodb", bufs=PRODB_BUFS) as prodbp,
            tc.tile_pool(name="small", bufs=4) as small,
            tc.tile_pool(name="tail", bufs=4) as tailp,
            tc.tile_pool(name="outp", bufs=4) as outp,
            tc.tile_pool(name="ps_ent", bufs=1, space="PSUM") as ps_ent,
            tc.tile_pool(name="ps_u", bufs=PSU_BUFS, space="PSUM") as ps_u,
            tc.tile_pool(name="ps_t", bufs=1, space="PSUM") as ps_t,
            tc.tile_pool(name="ps_agg", bufs=2, space="PSUM") as ps_agg,
            tc.tile_pool(name="ps_o", bufs=1, space="PSUM") as ps_o,
        ):
            # ---- resident loads (once) ----
            qT_sb = res.tile([D, NLOC], BF16)
            nc.sync.dma_start(out=qT_sb, in_=t_qT[:])
            entT_sb = [
                res.tile([128, NLOC], F32, tag="entT0", name="entT0_sb"),
                res.tile([128, NLOC], F32, tag="entT1", name="entT1_sb"),
                res.tile([45, NLOC], F32, tag="entT2", name="entT2_sb"),
            ]
            for sb, t in zip(entT_sb, (t_entT0, t_entT1, t_entT2)):
                nc.sync.dma_start(out=sb, in_=t[:])
            W2_sb = res.tile([D, OJ], BF16)
            nc.sync.dma_start(out=W2_sb, in_=t_W2[:])
            WpT_sb = [
                res.tile([128, D], F32, tag="WpT0", name="WpT0_sb"),
                res.tile([128, D], F32, tag="WpT1", name="WpT1_sb"),
                res.tile([45, D], F32, tag="WpT2", name="WpT2_sb"),
            ]
            for sb, t in zip(WpT_sb, (t_WpT0, t_WpT1, t_WpT2)):
                nc.sync.dma_start(out=sb, in_=t[:])
            WgT_sb = res.tile([D, D], F32)
            nc.sync.dma_start(out=WgT_sb, in_=t_WgT[:])
            WrT_sb = res.tile([D, 1], F32)
            nc.sync.dma_start(out=WrT_sb, in_=t_WrT[:])
            bbil_sb = res.tile([P, D], F32)
            nc.sync.dma_start(out=bbil_sb, in_=t_bbil[:])
            gb_sb = res.tile([D, 1], F32)
            nc.sync.dma_start(out=gb_sb, in_=t_gb[:])
            ident_sb = res.tile([P, P], F32)
            make_identity(nc, ident_sb)
            zeros_sb = res.tile([D, P], F32)
            nc.vector.memset(zeros_sb, 0.0)

            NCH = 5  # bilinear oj chunks
            CW = OJ // NCH  # 500
            O_PER = CW // D  # 10 o's per chunk

            def tree_reduce_blocks(buf, rows, nblk, width, out_ap):
                """Sum `nblk` contiguous blocks of `width` elems (axis -1 of
                buf[:rows]) via pairwise adds; final add writes out_ap (f32)."""
                cur = nblk
                while cur > 2:
                    lo = (cur + 1) // 2
                    hi = cur - lo
                    nc.vector.tensor_add(
                        buf[:rows, 0 : hi * width],
                        buf[:rows, 0 : hi * width],
                        buf[:rows, lo * width : (lo + hi) * width],
                    )
                    cur = lo
                if cur == 2:
                    nc.vector.tensor_add(
                        out_ap,
                        buf[:rows, 0:width],
                        buf[:rows, width : 2 * width],
                    )
                else:
                    nc.vector.tensor_copy(out_ap, buf[:rows, 0:width])

            def tree_reduce_inner(eng, v, out_ap, width):
                """v: [rows, nblk, width] view; sum inner axis into out_ap
                (f32 [rows, nblk]) on engine `eng`. Splits keep 4B align."""
                w = width
                while w > 2:
                    lo = (w // 2 + 1) // 2 * 2  # even split point >= w/2
                    hi = w - lo
                    eng.tensor_add(
                        v[:, :, 0:hi], v[:, :, 0:hi], v[:, :, lo : lo + hi]
                    )
                    w = lo
                eng.tensor_add(
                    out_ap.unsqueeze(2), v[:, :, 0:1], v[:, :, 1:2]
                )

            import contextlib

            rep_ctx = (
                tc.For_i(0, repeat, 1) if repeat > 1 else contextlib.nullcontext()
            )
            with rep_ctx:
              for it in range(N_TILES):
                r0 = it * P
                rows = min(P, NLOC - r0)
                rs = slice(r0, r0 + rows)

                nbr_t = nbrp.tile([P, K * D], nbr_dt)
                nc.sync.dma_start(out=nbr_t[:rows], in_=t_nbr[rs, :])
                sc_t = scp.tile([P, K + 1], nbr_dt)
                nc.sync.dma_start(out=sc_t[:rows], in_=t_sc[rs, :])
                s63_t = scp.tile([P, 1], F32, tag="s63")
                nc.sync.dma_start(out=s63_t[:rows], in_=t_sc63[rs, :])

                # --- entity projection on PE: ent[n,j] ---
                ent_ps = ps_ent.tile([P, D], F32)
                for c in range(3):
                    nc.tensor.matmul(
                        ent_ps[:rows],
                        entT_sb[c][:, rs],
                        WpT_sb[c],
                        start=(c == 0),
                        stop=(c == 2),
                    )
                ent_sb = small.tile([P, D], BF16, tag="ent")
                nc.scalar.copy(out=ent_sb[:rows], in_=ent_ps[:rows])

                # --- neighbor stage (nbr is d-major: nbr[n, d*K+k]) ---
                # d-range [0, ds) on DVE, [ds, D) on GPSIMD
                prodn = prodnp.tile([P, K * D], nbr_dt)
                pn = prodn[:rows].rearrange("p (d k) -> p d k", d=D)
                nb = nbr_t[:rows].rearrange("p (d k) -> p d k", d=D)
                scb = sc_t[:rows, 0:K].unsqueeze(1).broadcast_to([rows, D, K])
                wnbr = small.tile([P, D], F32, tag="wnbr")
                if ds > 0:
                    nc.vector.tensor_mul(
                        pn[:, 0:ds], nb[:, 0:ds], scb[:, 0:ds]
                    )
                    tree_reduce_inner(
                        nc.vector, pn[:, 0:ds], wnbr[:rows, 0:ds], K
                    )
                if ds < D:
                    nc.gpsimd.tensor_mul(
                        pn[:, ds:D], nb[:, ds:D], scb[:, ds:D]
                    )
                    tree_reduce_inner(
                        nc.gpsimd, pn[:, ds:D], wnbr[:rows, ds:D], K
                    )

                # --- bilinear stage ---
                prodb = prodbp.tile([P, OJ], BF16)
                usb = prodbp.tile([P, OJ], BF16, tag="usb")
                for c in range(NCH):
                    u_ps = ps_u.tile([P, CW], F32)
                    nc.tensor.matmul(
                        u_ps[:rows],
                        qT_sb[:, rs],
                        W2_sb[:, c * CW : (c + 1) * CW],
                        start=True,
                        stop=True,
                    )
                    nc.scalar.copy(
                        out=usb[:rows, c * CW : (c + 1) * CW], in_=u_ps[:rows]
                    )
                pb = prodb[:rows].rearrange("p (o j) -> p o j", o=D)
                ub = usb[:rows].rearrange("p (o j) -> p o j", o=D)
                eb = ent_sb[:rows].unsqueeze(1).broadcast_to([rows, D, D])
                noderaw = small.tile([P, D], F32, tag="noderaw")
                if os_ > 0:
                    nc.vector.tensor_mul(pb[:, 0:os_], ub[:, 0:os_], eb[:, 0:os_])
                    tree_reduce_inner(
                        nc.vector, pb[:, 0:os_], noderaw[:rows, 0:os_], D
                    )
                if os_ < D:
                    nc.gpsimd.tensor_mul(pb[:, os_:D], ub[:, os_:D], eb[:, os_:D])
                    tree_reduce_inner(
                        nc.gpsimd, pb[:, os_:D], noderaw[:rows, os_:D], D
                    )
                smeng = nc.gpsimd if smalls_pool else nc.vector
                nodeb = small.tile([P, D], F32, tag="nodeb")
                smeng.tensor_add(nodeb[:rows], noderaw[:rows], bbil_sb[:rows])

                # wtext = nodeb * s63 + wnbr
                wtext = small.tile([P, D], F32, tag="wtext")
                nc.vector.scalar_tensor_tensor(
                    out=wtext[:rows],
                    in0=nodeb[:rows],
                    scalar=s63_t[:rows],
                    in1=wnbr[:rows],
                    op0=OP.mult,
                    op1=OP.add,
                )

                # --- tail: agg = wtext @ Wg.T; feats=elu(agg+gb); out=feats@Wr.T+br
                wtT_ps = ps_t.tile([D, P], F32)
                nc.tensor.transpose(
                    wtT_ps[:, :rows], wtext[:rows], ident_sb[:rows, :rows]
                )
                wtT_sb = tailp.tile([D, P], F32, tag="wtT")
                nc.scalar.copy(out=wtT_sb[:, :rows], in_=wtT_ps[:, :rows])
                agg_ps = ps_agg.tile([D, P], F32)
                nc.tensor.matmul(
                    agg_ps[:, :rows], WgT_sb, wtT_sb[:, :rows], start=True, stop=True
                )
                e_sb = tailp.tile([D, P], F32, tag="e")
                nc.scalar.activation(
                    out=e_sb[:, :rows], in_=agg_ps[:, :rows], func=AF.Exp, bias=gb_sb
                )
                r_sb = tailp.tile([D, P], F32, tag="r")
                nc.scalar.activation(
                    out=r_sb[:, :rows], in_=agg_ps[:, :rows], func=AF.Relu, bias=gb_sb
                )
                feats = tailp.tile([D, P], F32, tag="feats")
                nc.vector.scalar_tensor_tensor(
                    out=feats[:, :rows],
                    in0=e_sb[:, :rows],
                    scalar=1.0,
                    in1=zeros_sb[:, :rows],
                    op0=OP.subtract,
                    op1=OP.min,
                )
                smeng.tensor_add(feats[:, :rows], feats[:, :rows], r_sb[:, :rows])
                out_ps = ps_o.tile([1, P], F32)
                nc.tensor.matmul(
                    out_ps[:, :rows], WrT_sb, feats[:, :rows], start=True, stop=True
                )
                out_sb = outp.tile([1, P], F32)
                nc.scalar.activation(
                    out=out_sb[:, :rows],
                    in_=out_ps[:, :rows],
                    func=AF.Identity,
                    bias=br_val,
                )
                nc.sync.dma_start(
                    out=t_out[rs, :].transpose([1, 0]), in_=out_sb[:, :rows]
                )

    nc.finalize()
    return nc


def kernel(
    query_emb,
    entity_emb,
    neighbor_embs,
    neighbor_scores,
    Wp,
    bp,
    Wbil,
    bbil,
    Wg,
    g_bias,
    Wr,
    br,
):
    nbr_np = _np_dt(NBR_DT)
    br_val = float(np.asarray(br).reshape(-1)[0])

    if "nc" not in _CACHE:
        _CACHE["nc"] = build_program(br_val)
    nc = _CACHE["nc"]

    # ---- shared weight prep ----
    # W2[i, o*D+j] = Wbil[o, i, j]
    W2 = np.ascontiguousarray(
        np.asarray(Wbil, np.float32).transpose(1, 0, 2).reshape(D, OJ)
    ).astype(ml_dtypes.bfloat16)
    WpT_aug = np.concatenate(
        [np.asarray(Wp, np.float32).T, np.asarray(bp, np.float32)[None, :]], axis=0
    )  # [301, 50]
    WpT_chunks = [
        np.ascontiguousarray(WpT_aug[0:128]),
        np.ascontiguousarray(WpT_aug[128:256]),
        np.ascontiguousarray(WpT_aug[256:301]),
    ]
    WgT = np.ascontiguousarray(np.asarray(Wg, np.float32).T)
    WrT = np.ascontiguousarray(np.asarray(Wr, np.float32).T)
    bbil_rep = np.ascontiguousarray(
        np.tile(np.asarray(bbil, np.float32)[None, :], (P, 1))
    )
    gb_col = np.ascontiguousarray(np.asarray(g_bias, np.float32)[:, None])

    q = np.asarray(query_emb, np.float32)
    ent = np.asarray(entity_emb, np.float32)
    nbr = np.asarray(neighbor_embs, np.float32)
    sc = np.asarray(neighbor_scores, np.float32)

    in_maps = []
    for c in range(N_CORES):
        s = slice(c * NLOC, (c + 1) * NLOC)
        ent_aug = np.concatenate(
            [ent[s], np.ones((NLOC, 1), np.float32)], axis=1
        ).T  # [301, NLOC]
        ent_aug = np.ascontiguousarray(ent_aug)
        in_maps.append(
            {
                "nbr": np.ascontiguousarray(
                    nbr[s].transpose(0, 2, 1).reshape(NLOC, K * D)
                ).astype(nbr_np),
                "scores": np.ascontiguousarray(sc[s]).astype(nbr_np),
                "s63": np.ascontiguousarray(sc[s, K : K + 1]),
                "qT": np.ascontiguousarray(q[s].T).astype(ml_dtypes.bfloat16),
                "entT0": np.ascontiguousarray(ent_aug[0:128]),
                "entT1": np.ascontiguousarray(ent_aug[128:256]),
                "entT2": np.ascontiguousarray(ent_aug[256:301]),
                "W2": W2,
                "WpT0": WpT_chunks[0],
                "WpT1": WpT_chunks[1],
                "WpT2": WpT_chunks[2],
                "WgT": WgT,
                "WrT": WrT,
                "bbil_rep": bbil_rep,
                "gbias_col": gb_col,
            }
        )

    _CACHE["last_in_maps"] = in_maps
    res = run_bass_kernel_spmd(nc, in_maps, core_ids=list(range(N_CORES)))
    out = np.concatenate([res.results[c]["out"] for c in range(N_CORES)], axis=0)
    return out.astype(np.float32)



# revision 55
# speedup vs baseline: 1.5160x; 1.5160x over previous
"""Trainium2 Bass kernel for NeuralECMModel (gnn_message_passing).

Math (per node n):
  ent  = entity_emb @ Wp.T + bp                                   [N,50]
  node = einsum('ni,oij,nj->no', q, Wbil, ent) + bbil             [N,50]
  wtext= sum_k s[n,k]*nbr[n,k,:] + s[n,63]*node[n,:]              [N,50]
  agg  = wtext @ Wg.T                                             [N,50]
  out  = elu(agg + g_bias) @ Wr.T + br                            [N,1]

Key restructuring (vs naive): Wg is folded into both branches so `agg`
is accumulated directly in PSUM by the PE:
  agg[p,n] = sum_{(d,k)} Wg[p,d]*s[n,k]*nbr[n,k,d]        (PE contraction
             over 25 chunks of the transposed neighbor stream)
           + s63[n]*(q Wtil[p] ent + bbilg[p])            (row-major bilinear,
             transpose-matmul-accumulated into the same PSUM tile)
  with Wtil[p,i,j] = sum_o Wg[p,o]*Wbil[o,i,j], bbilg = Wg @ bbil.

This removes the k-tree reduction from the vector engines entirely; the
score multiply is ONE full-rate bf16 DVE op per 500-node macro tile.

Sharding: pure data parallel over nodes, N=20000 -> 2500 nodes/core x 8.
"""

import numpy as np
import ml_dtypes

import concourse.bass as bass
import concourse.bacc as bacc
import concourse.tile as tile
import concourse.mybir as mybir
from concourse.bass_utils import run_bass_kernel_spmd
from concourse.masks import make_identity

F32 = mybir.dt.float32
BF16 = mybir.dt.bfloat16
OP = mybir.AluOpType
AF = mybir.ActivationFunctionType

N_CORES = 8
N = 20000
NLOC = N // N_CORES   # 2500
K = 63
D = 50
E = 300
EA = 304              # padded augmented entity rows (300 + ones + 3 zero)
P = 128
SR = 125              # bilinear sub-tile rows
# macro tile sizes: small ramp-up/ramp-down tiles shorten the DMA-bound
# startup and the drain at the end
MTS = (250, 250, 500, 500, 500, 500)
N_MT = len(MTS)
NCH = 25              # neighbor (d,k) chunks of 126 rows
CR = 2 * K            # 126 rows per chunk (2 d's x 63 k's)
OJ = D * D            # 2500

# bilinear o-split per sub-tile: DVE takes o in [0, osp), Pool [osp, D);
# cycling 40/30/30 averages ~33 despite the 10-o egress-chunk granularity
OSPS = (40, 30, 30)
# U psum chunk width in o's (each *D wide); one PSUM bank per chunk so every
# matmul output is bank-aligned (mid-bank matmul writes corrupt silently)
UW = 10
UCH = (10, 10, 10, 10, 10)

_CACHE = {}


def _tree_levels(w):
    """Pairwise-halving splits: [(hi, lo), ...] meaning x[0:hi] += x[lo:lo+hi]."""
    out = []
    while w > 1:
        lo = (w + 1) // 2
        hi = w - lo
        out.append((hi, lo))
        w = lo
    return out


def build_program(br_val: float, skip_bil=False, skip_nbr=False, dump_ng=False):
    nc = bacc.Bacc("TRN2", debug=False, num_devices=N_CORES)

    # ---- per-core inputs ----
    t_nbrT = nc.dram_tensor("nbrT", [NCH * CR, NLOC], BF16, kind="ExternalInput")
    # f32 pack: cols 0..19 s63 (col per sub-tile), col 20 rows 0..49 = g_bias
    t_f32p = nc.dram_tensor("f32p", [SR, NLOC // SR + 1], F32, kind="ExternalInput")
    # [50, 5000]: qT | W2til side by side
    t_qW = nc.dram_tensor("qW", [D, NLOC + OJ], BF16, kind="ExternalInput")
    t_entT0 = nc.dram_tensor("entT0", [128, NLOC], BF16, kind="ExternalInput")
    t_entT1 = nc.dram_tensor("entT1", [128, NLOC], BF16, kind="ExternalInput")
    t_entT2 = nc.dram_tensor("entT2", [EA - 256, NLOC], BF16, kind="ExternalInput")
    # [128, 201]: WpT0 | WpT1 | bbilg_rep+WrT | WpT2 (rows 0..47)
    t_wpack = nc.dram_tensor("wpack", [128, 100 + D + 1 + D], BF16, kind="ExternalInput")
    # [126, 3750]: sTrep | WgK
    t_sWgK = nc.dram_tensor("sWgK", [CR, NLOC + NCH * D], BF16, kind="ExternalInput")
    t_s63r = nc.dram_tensor("s63r", [1, NLOC], BF16, kind="ExternalInput")
    t_out = nc.dram_tensor("out", [1, NLOC], F32, kind="ExternalOutput")
    t_dbg = (
        nc.dram_tensor("dbg", [SR, (NLOC // SR) * D], BF16, kind="ExternalOutput")
        if dump_ng else None
    )

    with tile.TileContext(nc) as tc:
        with (
            tc.tile_pool(name="res", bufs=1) as res,
            tc.tile_pool(name="nbrp", bufs=2) as nbrp,
            tc.tile_pool(name="snbrp", bufs=2) as snbrp,
            tc.tile_pool(name="usbd", bufs=3) as usbd_p,
            tc.tile_pool(name="usbg", bufs=3) as usbg_p,
            tc.tile_pool(name="prodd", bufs=3) as prodd_p,
            tc.tile_pool(name="prodg", bufs=3) as prodg_p,
            tc.tile_pool(name="small", bufs=8) as small,
            tc.tile_pool(name="tailp", bufs=3) as tailp,
            tc.tile_pool(name="ps_agg", bufs=3, space="PSUM") as ps_agg,
            tc.tile_pool(name="ps_ent", bufs=1, space="PSUM") as ps_ent,
            tc.tile_pool(name="ps_u", bufs=3, space="PSUM") as ps_u,
            tc.tile_pool(name="ps_o", bufs=1, space="PSUM") as ps_o,
            nc.allow_low_precision(reason="bf16 tree-reduce; node term is ~10% of signal"),
        ):
            # ---- residents: bilinear-path deps first so PE/ACT/DVE can start
            # the sub-tile pipeline while the big neighbor stream lands.
            qW_sb = res.tile([D, NLOC + OJ], BF16)
            nc.sync.dma_start(out=qW_sb, in_=t_qW[:])
            wpack_sb = res.tile([P, 100 + D + 1 + D], BF16)
            nc.sync.dma_start(out=wpack_sb, in_=t_wpack[:])
            f32p_sb = res.tile([SR, NLOC // SR + 1], F32)
            nc.sync.dma_start(out=f32p_sb, in_=t_f32p[:])
            entT_sb = [
                res.tile([128, NLOC], BF16, tag="entT0", name="entT0_sb"),
                res.tile([128, NLOC], BF16, tag="entT1", name="entT1_sb"),
                res.tile([EA - 256, NLOC], BF16, tag="entT2", name="entT2_sb"),
            ]
            for sb, t in zip(entT_sb, (t_entT0, t_entT1, t_entT2)):
                nc.sync.dma_start(out=sb, in_=t[:])
            nbrT_v = t_nbrT[:].rearrange("(c p) n -> p c n", p=CR)

            nbr0 = nbrp.tile([CR, NCH * MTS[0]], BF16, tag="nbrt", name="nbr_t0")
            nc.sync.dma_start(
                out=nbr0.rearrange("p (c n) -> p c n", c=NCH),
                in_=nbrT_v[:, :, 0 : MTS[0]],
            )
            sWgK_sb = res.tile([CR, NLOC + NCH * D], BF16)
            nc.sync.dma_start(out=sWgK_sb, in_=t_sWgK[:])
            s63r_sb = res.tile([1, NLOC], BF16)
            nc.sync.dma_start(out=s63r_sb, in_=t_s63r[:])
            ident_sb = res.tile([P, P], BF16)
            make_identity(nc, ident_sb)
            out_row = res.tile([1, NLOC], F32)

            qT_sb = qW_sb[:, 0:NLOC]
            W2til_sb = qW_sb[:, NLOC : NLOC + OJ]
            WpT_sb = [
                wpack_sb[:, 0:D],
                wpack_sb[:, D : 2 * D],
                wpack_sb[0 : EA - 256, 151 : 151 + D],
            ]
            bbilg_rep = wpack_sb[:, 100:150]          # [128, 50]
            WrT_col = wpack_sb[0:D, 150:151]          # [50, 1]
            s63p_sb = f32p_sb[:, 0 : NLOC // SR]
            gb_sb = f32p_sb[0:D, NLOC // SR : NLOC // SR + 1]
            sTrep_sb = sWgK_sb[:, 0:NLOC]
            WgK_sb = sWgK_sb[:, NLOC : NLOC + NCH * D]

            n0 = 0
            gsub = 0
            for mt in range(N_MT):
                ntx = MTS[mt]
                ns = slice(n0, n0 + ntx)

                if mt == 0:
                    nbr_t = nbr0
                else:
                    nbr_t = nbrp.tile([CR, NCH * ntx], BF16, tag="nbrt", name=f"nbr_t{mt}")
                    nc.sync.dma_start(
                        out=nbr_t.rearrange("p (c n) -> p c n", c=NCH),
                        in_=nbrT_v[:, :, ns],
                    )
                nbr_v = nbr_t.rearrange("p (c n) -> p c n", c=NCH)
                aggC = ps_agg.tile([D, ntx], F32, tag="aggC", name="aggC")

                def emit_snbr():
                    snbr = snbrp.tile([CR, NCH * ntx], BF16, tag="snbr", name="snbr")
                    snbr_v = snbr.rearrange("p (c n) -> p c n", c=NCH)
                    sT_b = sTrep_sb[:, ns].unsqueeze(1).broadcast_to([CR, NCH, ntx])
                    nc.vector.tensor_mul(snbr_v, nbr_v, sT_b)
                    return snbr_v

                # scores multiply: ONE full-rate bf16 DVE op.  For mt 0 it is
                # emitted after the bilinear subs so DVE starts on work whose
                # inputs land first (the big neighbor DMA is still in flight).
                snbr_v = None
                if not skip_nbr and mt > 0:
                    snbr_v = emit_snbr()

                # ---- bilinear per 125-row sub-tile (row-major) ----
                node_gbs = []
                nsub = ntx // SR
                for st in range(nsub if not skip_bil else 0):
                    r0 = n0 + st * SR
                    rs = slice(r0, r0 + SR)
                    col = gsub + st

                    ent_ps = ps_ent.tile([SR, D], F32)
                    for c in range(3):
                        nc.tensor.matmul(
                            ent_ps,
                            entT_sb[c][:, rs],
                            WpT_sb[c],
                            start=(c == 0),
                            stop=(c == 2),
                        )
                    # PSUM->SBUF with the s63 per-node scale folded in
                    ents = small.tile([SR, D], BF16, tag="ents")
                    nc.scalar.activation(
                        out=ents,
                        in_=ent_ps,
                        func=AF.Copy,
                        scale=s63p_sb[:, col : col + 1],
                    )

                    # U = q @ W2til (chunks along o), egress to bf16 SBUF.
                    # DVE owns o in [0, OSP) (egress chunks 1-2), Pool owns
                    # [OSP, D) (chunk 3). Separate tiles per engine so their
                    # buffer rotations never couple.
                    osp = OSPS[(gsub + st) % len(OSPS)]
                    usbd = usbd_p.tile([SR, 40 * D], BF16)
                    usbg = usbg_p.tile([SR, 20 * D], BF16)
                    off = 0
                    for w in UCH:
                        u_ps = ps_u.tile([SR, w * D], F32)
                        nc.tensor.matmul(
                            u_ps,
                            qT_sb[:, rs],
                            W2til_sb[:, off * D : (off + w) * D],
                            start=True,
                            stop=True,
                        )
                        if off < osp:
                            nc.scalar.copy(
                                out=usbd[:, off * D : (off + w) * D], in_=u_ps
                            )
                        else:
                            nc.scalar.copy(
                                out=usbg[:, (off - osp) * D : (off - osp + w) * D],
                                in_=u_ps,
                            )
                        off += w

                    # prod = U * ents (broadcast over o); j-tree -> node_g
                    prodd = prodd_p.tile([SR, 40 * D], BF16)
                    prodg = prodg_p.tile([SR, 20 * D], BF16)
                    node_g = small.tile([SR, D], BF16, tag="node_g")
                    ev = ents.unsqueeze(1).broadcast_to([SR, D, D])
                    for eng, prod, usb, o0, o1 in (
                        (nc.vector, prodd, usbd, 0, osp),
                        (nc.gpsimd, prodg, usbg, osp, D),
                    ):
                        ow = o1 - o0
                        pvx = prod[:, 0 : ow * D].rearrange("p (o j) -> p o j", o=ow)
                        uvx = usb[:, 0 : ow * D].rearrange("p (o j) -> p o j", o=ow)
                        eng.tensor_mul(pvx, uvx, ev[:, o0:o1])
                        w = D
                        for hi, lo in _tree_levels(D):
                            if w == 2:
                                eng.tensor_add(
                                    node_g[:, o0:o1].unsqueeze(2),
                                    pvx[:, :, 0:1],
                                    pvx[:, :, 1:2],
                                )
                            else:
                                eng.tensor_add(
                                    pvx[:, :, 0:hi],
                                    pvx[:, :, 0:hi],
                                    pvx[:, :, lo : lo + hi],
                                )
                            w = lo

                    if dump_ng:
                        nc.sync.dma_start(
                            out=t_dbg[:, col * D : (col + 1) * D], in_=node_g
                        )
                    node_gbs.append(node_g)

                if not skip_nbr and snbr_v is None:
                    snbr_v = emit_snbr()

                # ---- neighbor contraction: agg[p, n] over 25 (d,k)-chunks ----
                for c in range(NCH if not skip_nbr else 0):
                    nc.tensor.matmul(
                        aggC,
                        WgK_sb[:, c * D : (c + 1) * D],
                        snbr_v[:, c, :],
                        start=(c == 0),
                        stop=False,
                    )
                # rank-1 bias term: agg += bbilg[o] * s63[n]
                nc.tensor.matmul(
                    aggC,
                    bbilg_rep[0:1],
                    s63r_sb[:, ns],
                    start=skip_nbr,
                    stop=skip_bil,
                )
                # ---- node contribution: transpose-accumulate ----
                for st in range(nsub if not skip_bil else 0):
                    nc.tensor.matmul(
                        aggC[:, st * SR : (st + 1) * SR],
                        node_gbs[st],
                        ident_sb[0:SR, 0:SR],
                        start=False,
                        stop=True,
                    )

                # ---- tail: elu(agg+gb) @ Wr + br ----
                e_sb = tailp.tile([D, ntx], BF16, tag="e")
                nc.scalar.activation(out=e_sb, in_=aggC, func=AF.Exp, bias=gb_sb)
                r_sb = tailp.tile([D, ntx], BF16, tag="r")
                nc.scalar.activation(out=r_sb, in_=aggC, func=AF.Relu, bias=gb_sb)
                # elu(x) = min(exp(x)-1, relu(x)) exactly
                feats = tailp.tile([D, ntx], BF16, tag="feats")
                nc.vector.scalar_tensor_tensor(
                    out=feats, in0=e_sb, scalar=-1.0, in1=r_sb,
                    op0=OP.add, op1=OP.min,
                )
                out_ps = ps_o.tile([1, ntx], F32, tag="out_ps", name="out_ps")
                nc.tensor.matmul(out_ps, WrT_col, feats, start=True, stop=True)
                nc.scalar.activation(
                    out=out_row[:, ns], in_=out_ps, func=AF.Identity, bias=br_val
                )
                n0 += ntx
                gsub += nsub

            nc.sync.dma_start(out=t_out[:], in_=out_row)

    nc.finalize()
    return nc


def kernel(
    query_emb,
    entity_emb,
    neighbor_embs,
    neighbor_scores,
    Wp,
    bp,
    Wbil,
    bbil,
    Wg,
    g_bias,
    Wr,
    br,
):
    br_val = float(np.asarray(br).reshape(-1)[0])
    if "nc" not in _CACHE:
        _CACHE["nc"] = build_program(br_val)
    nc = _CACHE["nc"]

    bf = ml_dtypes.bfloat16
    q = np.asarray(query_emb, np.float32)
    ent = np.asarray(entity_emb, np.float32)
    nbr = np.asarray(neighbor_embs, np.float32)
    sc = np.asarray(neighbor_scores, np.float32)
    Wg_ = np.asarray(Wg, np.float32)
    Wbil_ = np.asarray(Wbil, np.float32)

    # ---- shared weight prep ----
    # Wtil[p,i,j] = sum_o Wg[p,o] Wbil[o,i,j]; W2til[i, p*D+j] = Wtil[p,i,j]
    Wtil = np.einsum("po,oij->pij", Wg_, Wbil_)
    W2til_f = Wtil.transpose(1, 0, 2).reshape(D, OJ)
    bbilg = Wg_ @ np.asarray(bbil, np.float32)  # [50]
    WpT_aug = np.zeros((EA, D), np.float32)
    WpT_aug[0:E] = np.asarray(Wp, np.float32).T
    WpT_aug[E] = np.asarray(bp, np.float32)
    # WgK[(db,k), c*D+o] = Wg[o, 2c+db]
    WgT = Wg_.T  # [d, o]
    WgK = np.empty((CR, NCH * D), np.float32)
    for c in range(NCH):
        WgK[:, c * D : (c + 1) * D] = np.repeat(WgT[2 * c : 2 * c + 2], K, axis=0)
    # wpack: WpT0 | WpT1 | bbilg_rep+WrT | WpT2
    wpack = np.zeros((P, 100 + D + 1 + D), np.float32)
    wpack[:, 0:D] = WpT_aug[0:128]
    wpack[:, D : 2 * D] = WpT_aug[128:256]
    wpack[:, 100:150] = bbilg[None, :]
    wpack[0:D, 150] = np.asarray(Wr, np.float32).reshape(-1)
    wpack[0 : EA - 256, 151 : 151 + D] = WpT_aug[256:EA]
    wpack = wpack.astype(bf)
    gb = np.asarray(g_bias, np.float32)

    in_maps = []
    for c in range(N_CORES):
        s = slice(c * NLOC, (c + 1) * NLOC)
        ent_aug = np.zeros((EA, NLOC), np.float32)
        ent_aug[0:E] = ent[s].T
        ent_aug[E] = 1.0
        nbrT = nbr[s].transpose(2, 1, 0).reshape(NCH * CR, NLOC)
        sT = sc[s, 0:K].T  # [63, NLOC]
        f32p = np.zeros((SR, NLOC // SR + 1), np.float32)
        f32p[:, 0 : NLOC // SR] = sc[s, K].reshape(NLOC // SR, SR).T
        f32p[0:D, NLOC // SR] = gb
        s63r = sc[s, K][None, :]  # [1, NLOC]
        in_maps.append(
            {
                "nbrT": np.ascontiguousarray(nbrT).astype(bf),
                "sWgK": np.ascontiguousarray(
                    np.concatenate(
                        [np.concatenate([sT, sT], axis=0), WgK], axis=1
                    )
                ).astype(bf),
                "f32p": np.ascontiguousarray(f32p),
                "s63r": np.ascontiguousarray(s63r).astype(bf),
                "qW": np.ascontiguousarray(
                    np.concatenate([q[s].T, W2til_f], axis=1)
                ).astype(bf),
                "entT0": np.ascontiguousarray(ent_aug[0:128]).astype(bf),
                "entT1": np.ascontiguousarray(ent_aug[128:256]).astype(bf),
                "entT2": np.ascontiguousarray(ent_aug[256:EA]).astype(bf),
                "wpack": wpack,
            }
        )

    _CACHE["last_in_maps"] = in_maps
    res = run_bass_kernel_spmd(nc, in_maps, core_ids=list(range(N_CORES)))
    out = np.concatenate(
        [res.results[c]["out"].reshape(NLOC, 1) for c in range(N_CORES)], axis=0
    )
    return out.astype(np.float32)


# revision 65
# speedup vs baseline: 2.0454x; 1.3493x over previous
"""Trainium2 Bass kernel for NeuralECMModel (gnn_message_passing).

Math (per node n):
  ent  = entity_emb @ Wp.T + bp                                   [N,50]
  node = einsum('ni,oij,nj->no', q, Wbil, ent) + bbil             [N,50]
  wtext= sum_k s[n,k]*nbr[n,k,:] + s[n,63]*node[n,:]              [N,50]
  agg  = wtext @ Wg.T                                             [N,50]
  out  = elu(agg + g_bias) @ Wr.T + br                            [N,1]

Key restructuring (vs naive): Wg is folded into both branches so `agg`
is accumulated directly in PSUM by the PE:
  agg[p,n] = sum_{(d,k)} Wg[p,d]*s[n,k]*nbr[n,k,d]        (PE contraction
             over 25 chunks of the transposed neighbor stream)
           + s63[n]*(q Wtil[p] ent + bbilg[p])            (row-major bilinear,
             transpose-matmul-accumulated into the same PSUM tile)
  with Wtil[p,i,j] = sum_o Wg[p,o]*Wbil[o,i,j], bbilg = Wg @ bbil.

This removes the k-tree reduction from the vector engines entirely; the
score multiply is ONE full-rate bf16 DVE op per 500-node macro tile.

Sharding: pure data parallel over nodes, N=20000 -> 2500 nodes/core x 8.
"""

import numpy as np
import ml_dtypes

import concourse.bass as bass
import concourse.bacc as bacc
import concourse.tile as tile
import concourse.mybir as mybir
from concourse.bass_utils import run_bass_kernel_spmd
from concourse.masks import make_identity

F32 = mybir.dt.float32
BF16 = mybir.dt.bfloat16
OP = mybir.AluOpType
AF = mybir.ActivationFunctionType

N_CORES = 8
N = 20000
NLOC = N // N_CORES   # 2500
K = 63
D = 50
E = 300
EA = 304              # padded augmented entity rows (300 + ones + 3 zero)
P = 128
SR = 125              # bilinear sub-tile rows
# macro tile sizes: small ramp-up/ramp-down tiles shorten the DMA-bound
# startup and the drain at the end
MTS = (250, 250, 500, 500, 500, 500)
N_MT = len(MTS)
NCH = 25              # neighbor (d,k) chunks of 126 rows
NSPLIT = (9, 8, 8)    # neighbor chunk-group sizes (finer DMA/compute overlap)
CR = 2 * K            # 126 rows per chunk (2 d's x 63 k's)
OJ = D * D            # 2500

# bilinear o-split per sub-tile: DVE takes o in [0, osp), Pool [osp, D);
# cycling 40/30/30 averages ~33 despite the 10-o egress-chunk granularity
OSPS = (40, 30, 30)
# U psum chunk width in o's (each *D wide); one PSUM bank per chunk so every
# matmul output is bank-aligned (mid-bank matmul writes corrupt silently)
UW = 10
UCH = (10, 10, 10, 10, 10)

_CACHE = {}


def _tree_levels(w):
    """Pairwise-halving splits: [(hi, lo), ...] meaning x[0:hi] += x[lo:lo+hi]."""
    out = []
    while w > 1:
        lo = (w + 1) // 2
        hi = w - lo
        out.append((hi, lo))
        w = lo
    return out


def build_program(br_val: float, skip_bil=False, skip_nbr=False, dump_ng=False):
    nc = bacc.Bacc("TRN2", debug=False, num_devices=N_CORES)

    # ---- per-core inputs ----
    t_nbrT = nc.dram_tensor("nbrT", [NCH * CR, NLOC], BF16, kind="ExternalInput")
    # f32 pack: cols 0..19 s63 (col per sub-tile), col 20 rows 0..49 = g_bias
    t_f32p = nc.dram_tensor("f32p", [SR, NLOC // SR + 1], F32, kind="ExternalInput")
    # [50, 5000]: qT | W2til side by side
    t_qW = nc.dram_tensor("qW", [D, NLOC + OJ], BF16, kind="ExternalInput")
    t_entT0 = nc.dram_tensor("entT0", [128, NLOC], BF16, kind="ExternalInput")
    t_entT1 = nc.dram_tensor("entT1", [128, NLOC], BF16, kind="ExternalInput")
    t_entT2 = nc.dram_tensor("entT2", [EA - 256, NLOC], BF16, kind="ExternalInput")
    # [128, 201]: WpT0 | WpT1 | bbilg_rep+WrT | WpT2 (rows 0..47)
    t_wpack = nc.dram_tensor("wpack", [128, 100 + D + 1 + D], BF16, kind="ExternalInput")
    # [126, 3750]: sTrep | WgK
    t_sWgK = nc.dram_tensor("sWgK", [CR, NLOC + NCH * D], BF16, kind="ExternalInput")
    t_s63r = nc.dram_tensor("s63r", [1, NLOC], BF16, kind="ExternalInput")
    t_out = nc.dram_tensor("out", [1, NLOC], F32, kind="ExternalOutput")
    t_dbg = (
        nc.dram_tensor("dbg", [SR, (NLOC // SR) * D], BF16, kind="ExternalOutput")
        if dump_ng else None
    )

    with tile.TileContext(nc) as tc:
        with (
            tc.tile_pool(name="res", bufs=1) as res,
            tc.tile_pool(name="nbrp", bufs=2) as nbrp,
            tc.tile_pool(name="snbrp", bufs=2) as snbrp,
            tc.tile_pool(name="usbd", bufs=3) as usbd_p,
            tc.tile_pool(name="usbg", bufs=3) as usbg_p,
            tc.tile_pool(name="prodd", bufs=3) as prodd_p,
            tc.tile_pool(name="prodg", bufs=3) as prodg_p,
            tc.tile_pool(name="small", bufs=8) as small,
            tc.tile_pool(name="tailp", bufs=3) as tailp,
            tc.tile_pool(name="ps_agg", bufs=2, space="PSUM") as ps_agg,
            tc.tile_pool(name="ps_ent", bufs=2, space="PSUM") as ps_ent,
            tc.tile_pool(name="ps_u", bufs=3, space="PSUM") as ps_u,
            tc.tile_pool(name="ps_o", bufs=1, space="PSUM") as ps_o,
            nc.allow_low_precision(reason="bf16 tree-reduce; node term is ~10% of signal"),
        ):
            # ---- residents: bilinear-path deps first so PE/ACT/DVE can start
            # the sub-tile pipeline while the big neighbor stream lands.
            qW_sb = res.tile([D, NLOC + OJ], BF16)
            nc.sync.dma_start(out=qW_sb, in_=t_qW[:])
            wpack_sb = res.tile([P, 100 + D + 1 + D], BF16)
            nc.sync.dma_start(out=wpack_sb, in_=t_wpack[:])
            f32p_sb = res.tile([SR, NLOC // SR + 1], F32)
            nc.sync.dma_start(out=f32p_sb, in_=t_f32p[:])
            entT_sb = [
                res.tile([128, NLOC], BF16, tag="entT0", name="entT0_sb"),
                res.tile([128, NLOC], BF16, tag="entT1", name="entT1_sb"),
                res.tile([EA - 256, NLOC], BF16, tag="entT2", name="entT2_sb"),
            ]
            for sb, t in zip(entT_sb, (t_entT0, t_entT1, t_entT2)):
                nc.sync.dma_start(out=sb, in_=t[:])
            nbrT_v = t_nbrT[:].rearrange("(c p) n -> p c n", p=CR)

            nbr0_parts = []
            c0 = 0
            for gi, gw in enumerate(NSPLIT):
                t = nbrp.tile(
                    [CR, gw * MTS[0]], BF16, tag=f"nbr{gi}", name=f"nbr0_{gi}"
                )
                nc.sync.dma_start(
                    out=t.rearrange("p (c n) -> p c n", c=gw),
                    in_=nbrT_v[:, c0 : c0 + gw, 0 : MTS[0]],
                )
                nbr0_parts.append(t)
                c0 += gw
            sWgK_sb = res.tile([CR, NLOC + NCH * D], BF16)
            nc.sync.dma_start(out=sWgK_sb, in_=t_sWgK[:])
            s63r_sb = res.tile([1, NLOC], BF16)
            nc.sync.dma_start(out=s63r_sb, in_=t_s63r[:])
            ident_sb = res.tile([P, P], BF16)
            make_identity(nc, ident_sb)
            out_row = res.tile([1, NLOC], F32)

            qT_sb = qW_sb[:, 0:NLOC]
            W2til_sb = qW_sb[:, NLOC : NLOC + OJ]
            WpT_sb = [
                wpack_sb[:, 0:D],
                wpack_sb[:, D : 2 * D],
                wpack_sb[0 : EA - 256, 151 : 151 + D],
            ]
            bbilg_rep = wpack_sb[:, 100:150]          # [128, 50]
            WrT_col = wpack_sb[0:D, 150:151]          # [50, 1]
            s63p_sb = f32p_sb[:, 0 : NLOC // SR]
            gb_sb = f32p_sb[0:D, NLOC // SR : NLOC // SR + 1]
            sTrep_sb = sWgK_sb[:, 0:NLOC]
            WgK_sb = sWgK_sb[:, NLOC : NLOC + NCH * D]

            n0 = 0
            gsub = 0
            for mt in range(N_MT):
                ntx = MTS[mt]
                ns = slice(n0, n0 + ntx)

                if mt == 0:
                    nbr_parts = nbr0_parts
                else:
                    nbr_parts = []
                    c0 = 0
                    for gi, gw in enumerate(NSPLIT):
                        t = nbrp.tile(
                            [CR, gw * ntx], BF16, tag=f"nbr{gi}", name=f"nbr{mt}_{gi}"
                        )
                        nc.sync.dma_start(
                            out=t.rearrange("p (c n) -> p c n", c=gw),
                            in_=nbrT_v[:, c0 : c0 + gw, ns],
                        )
                        nbr_parts.append(t)
                        c0 += gw
                aggC = ps_agg.tile([D, ntx], F32, tag="aggC", name="aggC")

                def emit_snbr():
                    views = []
                    for gi, gw in enumerate(NSPLIT):
                        st_ = snbrp.tile(
                            [CR, gw * ntx], BF16, tag=f"snbr{gi}", name=f"snbr{gi}"
                        )
                        sv = st_.rearrange("p (c n) -> p c n", c=gw)
                        sT_bx = sTrep_sb[:, ns].unsqueeze(1).broadcast_to(
                            [CR, gw, ntx]
                        )
                        nc.vector.tensor_mul(
                            sv, nbr_parts[gi].rearrange("p (c n) -> p c n", c=gw), sT_bx
                        )
                        views.append(sv)
                    return views

                # scores multiply: ONE full-rate bf16 DVE op.  For mt 0 it is
                # emitted after the bilinear subs so DVE starts on work whose
                # inputs land first (the big neighbor DMA is still in flight).
                snbr_v = None
                if not skip_nbr and mt > 0:
                    snbr_v = emit_snbr()  # (sA, sB)

                # ---- bilinear per 125-row sub-tile (row-major) ----
                node_gbs = []
                nsub = ntx // SR
                for st in range(nsub if not skip_bil else 0):
                    r0 = n0 + st * SR
                    rs = slice(r0, r0 + SR)
                    col = gsub + st

                    ent_ps = ps_ent.tile([SR, D], F32)
                    for c in range(3):
                        nc.tensor.matmul(
                            ent_ps,
                            entT_sb[c][:, rs],
                            WpT_sb[c],
                            start=(c == 0),
                            stop=(c == 2),
                        )
                    # PSUM->SBUF with the s63 per-node scale folded in
                    ents = small.tile([SR, D], BF16, tag="ents")
                    nc.scalar.activation(
                        out=ents,
                        in_=ent_ps,
                        func=AF.Copy,
                        scale=s63p_sb[:, col : col + 1],
                    )

                    # U = q @ W2til (chunks along o), egress to bf16 SBUF.
                    # DVE owns o in [0, OSP) (egress chunks 1-2), Pool owns
                    # [OSP, D) (chunk 3). Separate tiles per engine so their
                    # buffer rotations never couple.
                    osp = OSPS[(gsub + st) % len(OSPS)]
                    usbd = usbd_p.tile([SR, 40 * D], BF16)
                    usbg = usbg_p.tile([SR, 20 * D], BF16)
                    off = 0
                    for w in UCH:
                        u_ps = ps_u.tile([SR, w * D], F32)
                        nc.tensor.matmul(
                            u_ps,
                            qT_sb[:, rs],
                            W2til_sb[:, off * D : (off + w) * D],
                            start=True,
                            stop=True,
                        )
                        if off < osp:
                            nc.scalar.copy(
                                out=usbd[:, off * D : (off + w) * D], in_=u_ps
                            )
                        else:
                            nc.scalar.copy(
                                out=usbg[:, (off - osp) * D : (off - osp + w) * D],
                                in_=u_ps,
                            )
                        off += w

                    # prod = U * ents (broadcast over o); j-tree -> node_g
                    prodd = prodd_p.tile([SR, 40 * D], BF16)
                    prodg = prodg_p.tile([SR, 20 * D], BF16)
                    node_g = small.tile([SR, D], BF16, tag="node_g")
                    ev = ents.unsqueeze(1).broadcast_to([SR, D, D])
                    for eng, prod, usb, o0, o1 in (
                        (nc.vector, prodd, usbd, 0, osp),
                        (nc.gpsimd, prodg, usbg, osp, D),
                    ):
                        ow = o1 - o0
                        pvx = prod[:, 0 : ow * D].rearrange("p (o j) -> p o j", o=ow)
                        uvx = usb[:, 0 : ow * D].rearrange("p (o j) -> p o j", o=ow)
                        eng.tensor_mul(pvx, uvx, ev[:, o0:o1])
                        w = D
                        for hi, lo in _tree_levels(D):
                            if w == 2:
                                eng.tensor_add(
                                    node_g[:, o0:o1].unsqueeze(2),
                                    pvx[:, :, 0:1],
                                    pvx[:, :, 1:2],
                                )
                            else:
                                eng.tensor_add(
                                    pvx[:, :, 0:hi],
                                    pvx[:, :, 0:hi],
                                    pvx[:, :, lo : lo + hi],
                                )
                            w = lo

                    if dump_ng:
                        nc.sync.dma_start(
                            out=t_dbg[:, col * D : (col + 1) * D], in_=node_g
                        )
                    node_gbs.append(node_g)

                if not skip_nbr and snbr_v is None:
                    snbr_v = emit_snbr()

                # ---- neighbor contraction: agg[p, n] over 25 (d,k)-chunks ----
                for c in range(NCH if not skip_nbr else 0):
                    gi, cc = 0, c
                    while cc >= NSPLIT[gi]:
                        cc -= NSPLIT[gi]
                        gi += 1
                    nc.tensor.matmul(
                        aggC,
                        WgK_sb[:, c * D : (c + 1) * D],
                        snbr_v[gi][:, cc, :],
                        start=(c == 0),
                        stop=False,
                    )
                # rank-1 bias term: agg += bbilg[o] * s63[n]
                nc.tensor.matmul(
                    aggC,
                    bbilg_rep[0:1],
                    s63r_sb[:, ns],
                    start=skip_nbr,
                    stop=skip_bil,
                )
                # ---- node contribution: transpose-accumulate ----
                for st in range(nsub if not skip_bil else 0):
                    nc.tensor.matmul(
                        aggC[:, st * SR : (st + 1) * SR],
                        node_gbs[st],
                        ident_sb[0:SR, 0:SR],
                        start=False,
                        stop=True,
                    )

                # ---- tail: elu(agg+gb) @ Wr + br ----
                e_sb = tailp.tile([D, ntx], BF16, tag="e")
                nc.scalar.activation(out=e_sb, in_=aggC, func=AF.Exp, bias=gb_sb)
                r_sb = tailp.tile([D, ntx], BF16, tag="r")
                nc.scalar.activation(out=r_sb, in_=aggC, func=AF.Relu, bias=gb_sb)
                # elu(x) = min(exp(x)-1, relu(x)) exactly
                feats = tailp.tile([D, ntx], BF16, tag="feats")
                nc.vector.scalar_tensor_tensor(
                    out=feats, in0=e_sb, scalar=-1.0, in1=r_sb,
                    op0=OP.add, op1=OP.min,
                )
                out_ps = ps_o.tile([1, ntx], F32, tag="out_ps", name="out_ps")
                nc.tensor.matmul(out_ps, WrT_col, feats, start=True, stop=True)
                nc.scalar.activation(
                    out=out_row[:, ns], in_=out_ps, func=AF.Identity, bias=br_val
                )
                n0 += ntx
                gsub += nsub

            nc.sync.dma_start(out=t_out[:], in_=out_row)

    nc.finalize()
    return nc


def kernel(
    query_emb,
    entity_emb,
    neighbor_embs,
    neighbor_scores,
    Wp,
    bp,
    Wbil,
    bbil,
    Wg,
    g_bias,
    Wr,
    br,
):
    br_val = float(np.asarray(br).reshape(-1)[0])
    if "nc" not in _CACHE:
        _CACHE["nc"] = build_program(br_val)
    nc = _CACHE["nc"]

    bf = ml_dtypes.bfloat16
    q = np.asarray(query_emb, np.float32)
    ent = np.asarray(entity_emb, np.float32)
    nbr = np.asarray(neighbor_embs, np.float32)
    sc = np.asarray(neighbor_scores, np.float32)
    Wg_ = np.asarray(Wg, np.float32)
    Wbil_ = np.asarray(Wbil, np.float32)

    # ---- shared weight prep ----
    # Wtil[p,i,j] = sum_o Wg[p,o] Wbil[o,i,j]; W2til[i, p*D+j] = Wtil[p,i,j]
    Wtil = np.einsum("po,oij->pij", Wg_, Wbil_)
    W2til_f = Wtil.transpose(1, 0, 2).reshape(D, OJ)
    bbilg = Wg_ @ np.asarray(bbil, np.float32)  # [50]
    WpT_aug = np.zeros((EA, D), np.float32)
    WpT_aug[0:E] = np.asarray(Wp, np.float32).T
    WpT_aug[E] = np.asarray(bp, np.float32)
    # WgK[(db,k), c*D+o] = Wg[o, 2c+db]
    WgT = Wg_.T  # [d, o]
    WgK = np.empty((CR, NCH * D), np.float32)
    for c in range(NCH):
        WgK[:, c * D : (c + 1) * D] = np.repeat(WgT[2 * c : 2 * c + 2], K, axis=0)
    # wpack: WpT0 | WpT1 | bbilg_rep+WrT | WpT2
    wpack = np.zeros((P, 100 + D + 1 + D), np.float32)
    wpack[:, 0:D] = WpT_aug[0:128]
    wpack[:, D : 2 * D] = WpT_aug[128:256]
    wpack[:, 100:150] = bbilg[None, :]
    wpack[0:D, 150] = np.asarray(Wr, np.float32).reshape(-1)
    wpack[0 : EA - 256, 151 : 151 + D] = WpT_aug[256:EA]
    wpack = wpack.astype(bf)
    gb = np.asarray(g_bias, np.float32)

    in_maps = []
    for c in range(N_CORES):
        s = slice(c * NLOC, (c + 1) * NLOC)
        ent_aug = np.zeros((EA, NLOC), np.float32)
        ent_aug[0:E] = ent[s].T
        ent_aug[E] = 1.0
        nbrT = nbr[s].transpose(2, 1, 0).reshape(NCH * CR, NLOC)
        sT = sc[s, 0:K].T  # [63, NLOC]
        f32p = np.zeros((SR, NLOC // SR + 1), np.float32)
        f32p[:, 0 : NLOC // SR] = sc[s, K].reshape(NLOC // SR, SR).T
        f32p[0:D, NLOC // SR] = gb
        s63r = sc[s, K][None, :]  # [1, NLOC]
        in_maps.append(
            {
                "nbrT": np.ascontiguousarray(nbrT).astype(bf),
                "sWgK": np.ascontiguousarray(
                    np.concatenate(
                        [np.concatenate([sT, sT], axis=0), WgK], axis=1
                    )
                ).astype(bf),
                "f32p": np.ascontiguousarray(f32p),
                "s63r": np.ascontiguousarray(s63r).astype(bf),
                "qW": np.ascontiguousarray(
                    np.concatenate([q[s].T, W2til_f], axis=1)
                ).astype(bf),
                "entT0": np.ascontiguousarray(ent_aug[0:128]).astype(bf),
                "entT1": np.ascontiguousarray(ent_aug[128:256]).astype(bf),
                "entT2": np.ascontiguousarray(ent_aug[256:EA]).astype(bf),
                "wpack": wpack,
            }
        )

    _CACHE["last_in_maps"] = in_maps
    res = run_bass_kernel_spmd(nc, in_maps, core_ids=list(range(N_CORES)))
    out = np.concatenate(
        [res.results[c]["out"].reshape(NLOC, 1) for c in range(N_CORES)], axis=0
    )
    return out.astype(np.float32)
